# revision 20
# baseline (speedup 1.0000x reference)
"""Trainium2 Bass kernel for the KAN classifier (tanh-basis MLP).

logits = sigmoid(tanh((x[:,:,None]-centers)*scales).reshape(B,-1) @ Wb + bb) @ Wh + bh

Sharding: data-parallel over batch across 8 NeuronCores (512 rows each).
Per core: basis expansion on ScalarE (tanh LUT with per-partition scale/bias,
fp8 output, computed once in group 0 and cached in SBUF), the big matmul on
TensorE in fp8 DoubleRow mode (256-deep contraction per matmul, ~2x PE rate),
the head matmul in bf16 with fp32 PSUM accumulation.

Groups 1-3 issue their matmuls in a skewed "wavefront" order so that PSUM
bank h's first matmul trails bank h-1 by one chunk: the 8 serial sigmoid
evictions of the previous group then stay off the PE critical path.
"""

import sys

sys.path.insert(0, "/opt/trn_rl_repo")

import ml_dtypes
import numpy as np

import concourse.bass as bass
import concourse.mybir as mybir
import concourse.tile as tile
from concourse.bass_utils import run_bass_kernel_spmd
from concourse.vector_clock import ScopedClock

IN_DIM, HIDDEN, CLASSES, NBASIS, B = 1024, 4096, 1000, 16, 4096
NCORES = 8
BL = B // NCORES          # 512 batch rows per core
K = IN_DIM * NBASIS       # 16384 contraction dim (reordered j*IN_DIM + d)
KC = K // 128             # 128 K-chunks (tanh/scale/bias granularity)
KC2 = KC // 2             # 64 K-pair-chunks (one DoubleRow matmul each)
NG = 4                    # hidden groups (8 psum banks each)
HT = HIDDEN // 128        # 32 hidden tiles
CH = CLASSES // 2         # 500 logits per psum half
ALPHA = 2048.0            # fp8 weight scale (undone in the sigmoid)

F32 = mybir.dt.float32
BF16 = mybir.dt.bfloat16
FP8 = mybir.dt.float8e4
AF = mybir.ActivationFunctionType
DR = mybir.MatmulPerfMode.DoubleRow


def _patched_drain_and_barrier(self, tick_clock, wait_clock):
    # The walrus build in this image caps sync-waits per CTRL instruction;
    # stock Tile piles one wait per live semaphore onto the single tail
    # Drain. Re-emit them as standalone single-wait instructions.
    nc = self.nc
    drain_inst = nc.sync.drain()
    wait_clock.add_sem_waits(
        drain_inst.ins, ScopedClock({None: tick_clock.global_clock})
    )
    si = drain_inst.ins.sync_info
    waits = list(si.on_wait)
    if len(waits) > 2:
        si.on_wait = []
        handles = {h.num: h for h in self.sems.allocated().values()}
        for w in waits:
            nc.sync.wait_ge(handles[w.id], w.wait_value)
    nc.all_engine_barrier()
    popped = nc._tile_sem_poison_stack.pop()
    assert popped is self._sem_poison
    nc.clear_and_free_semaphores(list(self.sems.allocated().values()))
    nc.all_engine_barrier()


tile.TileContext._drain_and_barrier = _patched_drain_and_barrier

# Walrus also rejects >2 sync-waits on ANY instruction. Post-process the
# serialized BIR: hoist excess waits onto EventSemaphore instructions emitted
# immediately before, on the same engine (engine streams are in-order, so a
# prior standalone wait is equivalent).
_MAXW = 1


def _split_excess_waits(raw: bytes) -> bytes:
    import orjson

    m = orjson.loads(raw)
    n_new = 0
    for fn in m.get("functions", []):
        for bb in fn.get("blocks", []):
            insts = bb.get("instructions", [])
            if not any(
                len((i.get("sync_info") or {}).get("on_wait") or []) > _MAXW
                for i in insts
            ):
                continue
            out = []
            for ins in insts:
                si = ins.get("sync_info")
                ow = (si or {}).get("on_wait") or []
                if len(ow) > _MAXW:
                    imm = [w for w in ow if not w.get("wait_reg")]
                    reg = [w for w in ow if w.get("wait_reg")]
                    assert len(reg) <= _MAXW, "too many register waits"
                    n_hoist = len(ow) - _MAXW
                    hoisted, kept = imm[:n_hoist], imm[n_hoist:] + reg
                    for w in hoisted:
                        n_new += 1
                        out.append(
                            {
                                "debug": ins.get("debug"),
                                "engine": ins["engine"],
                                "ins": [],
                                "name": f"WSPLIT-{n_new}",
                                "opcode": "EventSemaphore",
                                "outs": [],
                                "sync_info": {"on_update": [], "on_wait": [w]},
                            }
                        )
                    si["on_wait"] = kept
                out.append(ins)
            bb["instructions"] = out
    return orjson.dumps(m)


_orig_to_json_bytes = bass.Bass.to_json_bytes


def _to_json_bytes_split(self, *a, **kw):
    return _split_excess_waits(_orig_to_json_bytes(self, *a, **kw))


bass.Bass.to_json_bytes = _to_json_bytes_split


def build_program() -> bass.Bass:
    nc = bass.Bass()
    xt = nc.declare_dram_parameter("xt", [8, 128, BL], BF16, isOutput=False)
    scl = nc.declare_dram_parameter("scl", [128, KC], F32, isOutput=False)
    bia = nc.declare_dram_parameter("bia", [128, KC], F32, isOutput=False)
    wt = nc.declare_dram_parameter(
        "wt", [NG, KC2, 128, 2, 8 * 128], FP8, isOutput=False
    )
    bbp = nc.declare_dram_parameter("bb", [128, HT], F32, isOutput=False)
    wh = nc.declare_dram_parameter("wh", [HT, 128, CLASSES], BF16, isOutput=False)
    bh = nc.declare_dram_parameter("bh", [1, CLASSES], BF16, isOutput=False)
    out = nc.declare_dram_parameter("out", [BL, CLASSES], F32, isOutput=True)

    with tile.TileContext(nc) as tc:
        with (
            tc.tile_pool(name="consts", bufs=1) as consts,
            # 24-deep wt ring: the 8-core-aggregate HBM stream runs ~4%
            # short of the PE's consumption rate; the surplus banked into
            # the ring during the ~11us startup covers most of the
            # cumulative deficit so the PE doesn't stall on a beat.
            tc.tile_pool(name="wpool", bufs=24) as wpool,
            tc.tile_pool(name="whpool", bufs=8) as whpool,
            tc.tile_pool(name="apool", bufs=4) as apool,
            tc.tile_pool(name="opool", bufs=2) as opool,
            tc.tile_pool(name="psp", bufs=1, space="PSUM") as psp,
        ):
            # Warm the ACT table: sigmoid_and_others contains both tanh and
            # sigmoid, so the one ~2.7us table load happens under the
            # startup DMAs and never again.
            warm = consts.tile([1, 8], F32)
            nc.vector.memset(warm, 0.0)
            warm2 = consts.tile([1, 8], F32)
            nc.scalar.activation(warm2, warm, AF.Sigmoid)

            # startup-critical DMAs: scl/bia/bb/bh on the (otherwise idle)
            # scalar queue, xt blocks on gpsimd, so sync's first wt chunks
            # are never queued behind them
            scl_sb = consts.tile([128, KC], F32)
            nc.scalar.dma_start(out=scl_sb, in_=scl[:, :])
            bia_sb = consts.tile([128, KC], F32)
            nc.scalar.dma_start(out=bia_sb, in_=bia[:, :])
            xt_sb = consts.tile([128, 8, BL], BF16)
            for d in range(8):
                nc.gpsimd.dma_start(out=xt_sb[:, d, :], in_=xt[d])

            # tanh basis values, computed once in group 0 and reused by
            # groups 1-3 (fp8, 64KB/partition)
            feats = consts.tile([128, KC, BL], FP8)
            phi = consts.tile([128, HT, BL], BF16)
            bb_sb = consts.tile([128, HT], F32)
            nc.scalar.dma_start(out=bb_sb, in_=bbp[:, :])
            bh_sb = consts.tile([1, CLASSES], BF16)
            nc.scalar.dma_start(out=bh_sb, in_=bh[:, :])
            ones_sb = consts.tile([1, 128], BF16)
            nc.vector.memset(ones_sb, 1.0)

            # PE clock warmup: the HAM throttle releases only after ~3.4us
            # of sustained matmul activity. A dozen dummy matmuls (no DMA
            # deps, so they run during the startup DMA wait) get the PE to
            # 2.4GHz before the first real matmul.
            zrhs = consts.tile([1, BL], BF16)
            nc.vector.memset(zrhs, 0.0)
            warm_ps = psp.tile([128, BL], F32, tag="ps0", name="warm_ps")
            for w in range(12):
                nc.tensor.matmul(
                    warm_ps,
                    lhsT=ones_sb,
                    rhs=zrhs,
                    start=(w == 0),
                    stop=(w == 11),
                )

            wh_sb = {}

            def load_wh(i, eng):
                t = whpool.tile([128, CLASSES], BF16, name=f"wh_{i}", tag="wh")
                eng.dma_start(out=t, in_=wh[i])
                wh_sb[i] = t

            # ---- phase 1: h^T = Wb'.T-chunks @ feats (fp8 DoubleRow),
            #      phi = sigmoid(h/ALPHA + bb)
            for g in range(NG):
                psums = [
                    psp.tile([128, BL], F32, tag=f"ps{h}", name=f"ps_{g}_{h}")
                    for h in range(8)
                ]
                if g == 0:
                    # sequential order; tanh computed just-in-time.
                    # DVE precomputes the per-chunk args s*x+b so ScalarE
                    # needs only ONE big-N ACT per pair-chunk: two N=512
                    # ACTs (2*1732ns) would run slightly slower than the
                    # 8 matmuls (1728ns) and pace the PE; one N=1024 ACT
                    # (~1200ns) leaves 30% ScalarE slack.
                    for kc in range(KC2):
                        wtile = wpool.tile([128, 2, 8 * 128], FP8, tag="wt")
                        if kc < 8:
                            eng = nc.sync
                        else:
                            eng = (nc.sync, nc.gpsimd, nc.scalar)[kc % 3]
                        eng.dma_start(out=wtile, in_=wt[g, kc])
                        arg = apool.tile(
                            [128, 2, BL], BF16, name=f"arg_{kc}", tag="arg"
                        )
                        for j in (0, 1):
                            m = 2 * kc + j
                            nc.vector.tensor_scalar(
                                arg[:, j, :],
                                xt_sb[:, m % 8, :],
                                scl_sb[:, m : m + 1],
                                bia_sb[:, m : m + 1],
                                mybir.AluOpType.mult,
                                mybir.AluOpType.add,
                            )
                        nc.scalar.activation(
                            feats[:, 2 * kc : 2 * kc + 2, :], arg, AF.Tanh
                        )
                        for h in range(8):
                            nc.tensor.matmul(
                                psums[h],
                                lhsT=wtile[:, :, h * 128 : (h + 1) * 128],
                                rhs=feats[:, 2 * kc : 2 * kc + 2, :],
                                start=(kc == 0),
                                stop=(kc == KC2 - 1),
                                perf_mode=DR,
                            )
                else:
                    # wavefront: bank h runs chunk s-h at slot s, so bank h's
                    # first matmul comes ~h*2us after the previous group's
                    # last — covering the previous group's sigmoid tail.
                    wt_sb = {}
                    for s in range(KC2 + 7):
                        if s < KC2:
                            wtile = wpool.tile(
                                [128, 2, 8 * 128], FP8, name=f"wt_{g}_{s}",
                                tag="wt",
                            )
                            # 3-queue rotation (ScalarE is idle in groups
                            # 1-3 apart from the 8 sigmoids, which are
                            # emitted before the first scalar-issued chunk
                            # is needed): 2 queues alone run ~4% below the
                            # PE's consumption rate and stall it on a beat.
                            if s < 8:
                                eng = nc.sync if s % 2 == 0 else nc.gpsimd
                            else:
                                eng = (nc.sync, nc.gpsimd, nc.scalar)[s % 3]
                            eng.dma_start(out=wtile, in_=wt[g, s])
                            wt_sb[s] = wtile
                        elif g == 3:
                            # prefetch the first 8 Wh tiles in the wavefront
                            # tail slots (s=64..71): the wt stream is done,
                            # so these 2MB don't compete with it. The
                            # tile_wait_until anchor stops the scheduler
                            # from hoisting these no-dep DMAs to startup.
                            with tc.tile_wait_until(0.21):
                                load_wh(
                                    s - KC2,
                                    nc.gpsimd if s % 2 == 0 else nc.sync,
                                )
                        for h in range(8):
                            kc = s - h
                            if 0 <= kc < KC2:
                                nc.tensor.matmul(
                                    psums[h],
                                    lhsT=wt_sb[kc][:, :, h * 128 : (h + 1) * 128],
                                    rhs=feats[:, 2 * kc : 2 * kc + 2, :],
                                    start=(kc == 0),
                                    stop=(kc == KC2 - 1),
                                    perf_mode=DR,
                                )
                for h in range(8):
                    i = g * 8 + h
                    nc.scalar.activation(
                        phi[:, i, :],
                        psums[h],
                        AF.Sigmoid,
                        bias=bb_sb[:, i : i + 1],
                        scale=1.0 / ALPHA,
                    )

            # ---- phase 2: logits = phi.T @ Wh + bh  (bf16, single pass,
            #      all 8 psum banks, Wh streamed through an 8-deep ring)
            ps2 = {}
            for b4 in range(4):
                for c2 in range(2):
                    idx = b4 * 2 + c2
                    ps = psp.tile(
                        [128, CH], F32, tag=f"ps{idx}", name=f"q_{b4}_{c2}"
                    )
                    ps2[(b4, c2)] = ps
                    # bias init: psum = ones^T @ bh_slice (K=1 matmul)
                    nc.tensor.matmul(
                        ps,
                        lhsT=ones_sb,
                        rhs=bh_sb[:, c2 * CH : (c2 + 1) * CH],
                        start=True,
                        stop=False,
                    )
            for i in range(HT):
                if i + 7 < HT:
                    load_wh(i + 7, nc.sync if i % 2 == 0 else nc.gpsimd)
                for b4 in range(4):
                    for c2 in range(2):
                        nc.tensor.matmul(
                            ps2[(b4, c2)],
                            lhsT=phi[:, i, b4 * 128 : (b4 + 1) * 128],
                            rhs=wh_sb[i][:, c2 * CH : (c2 + 1) * CH],
                            start=False,
                            stop=(i == HT - 1),
                        )
            # eviction tail: copy each psum half out on alternating engines
            # (DVE + ScalarE) and DMA each half separately over 3 queues so
            # the tail after the last matmul is a few us, not 15.
            for b4 in range(4):
                out_sb = opool.tile(
                    [128, CLASSES], F32, name=f"out_sb_{b4}", tag="out"
                )
                for c2 in range(2):
                    p = b4 * 2 + c2
                    dst = out_sb[:, c2 * CH : (c2 + 1) * CH]
                    if p % 2 == 0:
                        nc.vector.tensor_copy(dst, ps2[(b4, c2)])
                    else:
                        nc.scalar.activation(dst, ps2[(b4, c2)], AF.Copy)
                    (nc.sync, nc.gpsimd, nc.scalar)[p % 3].dma_start(
                        out=out[
                            b4 * 128 : (b4 + 1) * 128,
                            c2 * CH : (c2 + 1) * CH,
                        ],
                        in_=dst,
                    )
    return nc


_CACHE: dict = {}


def _prep_inputs(x, centers, scales, Wb, bb, Wh, bh):
    bf16 = ml_dtypes.bfloat16
    fp8 = ml_dtypes.float8_e4m3
    # K reorder: k' = j*IN_DIM + d  (so a 128-chunk shares one (d-block, j))
    scale_vec = np.ascontiguousarray(scales.T).reshape(K)
    bias_vec = np.ascontiguousarray(-(scales * centers).T).reshape(K)
    scl = np.ascontiguousarray(scale_vec.reshape(KC, 128).T).astype(np.float32)
    bia = np.ascontiguousarray(bias_vec.reshape(KC, 128).T).astype(np.float32)
    # Wb rows permuted to k' order, scaled by ALPHA, tiled
    # [g, kc, ki, pair, h*128+c] for DoubleRow (pair = second 128 of each
    # 256-deep contraction chunk)
    Wbp = (
        Wb.reshape(IN_DIM, NBASIS, HIDDEN)
        .transpose(1, 0, 2)
        .reshape(K, HIDDEN)
    )
    wt = np.ascontiguousarray(
        (Wbp * ALPHA)
        .reshape(KC2, 2, 128, NG, 8, 128)
        .transpose(3, 0, 2, 1, 4, 5)
        .reshape(NG, KC2, 128, 2, 8 * 128)
    ).astype(fp8)
    bbp = np.ascontiguousarray(bb.reshape(HT, 128).T).astype(np.float32)
    whp = np.ascontiguousarray(Wh.reshape(HT, 128, CLASSES)).astype(bf16)
    bhp = np.ascontiguousarray(bh.reshape(1, CLASSES)).astype(bf16)
    xT = np.ascontiguousarray(x.T)  # [IN_DIM, B]
    in_maps = []
    for c in range(NCORES):
        xt_c = (
            np.ascontiguousarray(xT[:, c * BL : (c + 1) * BL])
            .reshape(8, 128, BL)
            .astype(bf16)
        )
        in_maps.append(
            {
                "xt": xt_c,
                "scl": scl,
                "bia": bia,
                "wt": wt,
                "bb": bbp,
                "wh": whp,
                "bh": bhp,
            }
        )
    return in_maps


def kernel(x, centers, scales, Wb, bb, Wh, bh):
    x = np.asarray(x, dtype=np.float32)
    centers = np.asarray(centers, dtype=np.float32)
    scales = np.asarray(scales, dtype=np.float32)
    Wb = np.asarray(Wb, dtype=np.float32)
    bb = np.asarray(bb, dtype=np.float32)
    Wh = np.asarray(Wh, dtype=np.float32)
    bh = np.asarray(bh, dtype=np.float32)

    if "nc" not in _CACHE:
        _CACHE["nc"] = build_program()
    nc = _CACHE["nc"]
    in_maps = _prep_inputs(x, centers, scales, Wb, bb, Wh, bh)
    res = run_bass_kernel_spmd(nc, in_maps, list(range(NCORES)))
    return np.concatenate(
        [res.results[c]["out"] for c in range(NCORES)], axis=0
    )


# revision 24
# speedup vs baseline: 1.0034x; 1.0034x over previous
"""Trainium2 Bass kernel for the KAN classifier (tanh-basis MLP).

logits = sigmoid(tanh((x[:,:,None]-centers)*scales).reshape(B,-1) @ Wb + bb) @ Wh + bh

Sharding: data-parallel over batch across 8 NeuronCores (512 rows each).
Per core: basis expansion on ScalarE (tanh LUT with per-partition scale/bias,
fp8 output, computed once in group 0 and cached in SBUF), the big matmul on
TensorE in fp8 DoubleRow mode (256-deep contraction per matmul, ~2x PE rate),
the head matmul in bf16 with fp32 PSUM accumulation.

Groups 1-3 issue their matmuls in a skewed "wavefront" order so that PSUM
bank h's first matmul trails bank h-1 by one chunk: the 8 serial sigmoid
evictions of the previous group then stay off the PE critical path.
"""

import sys

sys.path.insert(0, "/opt/trn_rl_repo")

import ml_dtypes
import numpy as np

import concourse.bass as bass
import concourse.mybir as mybir
import concourse.tile as tile
from concourse.bass_utils import run_bass_kernel_spmd
from concourse.vector_clock import ScopedClock

IN_DIM, HIDDEN, CLASSES, NBASIS, B = 1024, 4096, 1000, 16, 4096
NCORES = 8
BL = B // NCORES          # 512 batch rows per core
K = IN_DIM * NBASIS       # 16384 contraction dim (reordered j*IN_DIM + d)
KC = K // 128             # 128 K-chunks (tanh/scale/bias granularity)
KC2 = KC // 2             # 64 K-pair-chunks (one DoubleRow matmul each)
NG = 4                    # hidden groups (8 psum banks each)
HT = HIDDEN // 128        # 32 hidden tiles
CH = CLASSES // 2         # 500 logits per psum half
ALPHA = 2048.0            # fp8 weight scale (undone in the sigmoid)

F32 = mybir.dt.float32
BF16 = mybir.dt.bfloat16
FP8 = mybir.dt.float8e4
AF = mybir.ActivationFunctionType
DR = mybir.MatmulPerfMode.DoubleRow


def _patched_drain_and_barrier(self, tick_clock, wait_clock):
    # The walrus build in this image caps sync-waits per CTRL instruction;
    # stock Tile piles one wait per live semaphore onto the single tail
    # Drain. Re-emit them as standalone single-wait instructions.
    nc = self.nc
    drain_inst = nc.sync.drain()
    wait_clock.add_sem_waits(
        drain_inst.ins, ScopedClock({None: tick_clock.global_clock})
    )
    si = drain_inst.ins.sync_info
    waits = list(si.on_wait)
    if len(waits) > 2:
        si.on_wait = []
        handles = {h.num: h for h in self.sems.allocated().values()}
        for w in waits:
            nc.sync.wait_ge(handles[w.id], w.wait_value)
    nc.all_engine_barrier()
    popped = nc._tile_sem_poison_stack.pop()
    assert popped is self._sem_poison
    nc.clear_and_free_semaphores(list(self.sems.allocated().values()))
    nc.all_engine_barrier()


tile.TileContext._drain_and_barrier = _patched_drain_and_barrier

# Walrus also rejects >2 sync-waits on ANY instruction. Post-process the
# serialized BIR: hoist excess waits onto EventSemaphore instructions emitted
# immediately before, on the same engine (engine streams are in-order, so a
# prior standalone wait is equivalent).
_MAXW = 1


def _split_excess_waits(raw: bytes) -> bytes:
    import orjson

    m = orjson.loads(raw)
    n_new = 0
    for fn in m.get("functions", []):
        for bb in fn.get("blocks", []):
            insts = bb.get("instructions", [])
            if not any(
                len((i.get("sync_info") or {}).get("on_wait") or []) > _MAXW
                for i in insts
            ):
                continue
            out = []
            for ins in insts:
                si = ins.get("sync_info")
                ow = (si or {}).get("on_wait") or []
                if len(ow) > _MAXW:
                    imm = [w for w in ow if not w.get("wait_reg")]
                    reg = [w for w in ow if w.get("wait_reg")]
                    assert len(reg) <= _MAXW, "too many register waits"
                    n_hoist = len(ow) - _MAXW
                    hoisted, kept = imm[:n_hoist], imm[n_hoist:] + reg
                    for w in hoisted:
                        n_new += 1
                        out.append(
                            {
                                "debug": ins.get("debug"),
                                "engine": ins["engine"],
                                "ins": [],
                                "name": f"WSPLIT-{n_new}",
                                "opcode": "EventSemaphore",
                                "outs": [],
                                "sync_info": {"on_update": [], "on_wait": [w]},
                            }
                        )
                    si["on_wait"] = kept
                out.append(ins)
            bb["instructions"] = out
    return orjson.dumps(m)


_orig_to_json_bytes = bass.Bass.to_json_bytes


def _to_json_bytes_split(self, *a, **kw):
    return _split_excess_waits(_orig_to_json_bytes(self, *a, **kw))


bass.Bass.to_json_bytes = _to_json_bytes_split


def build_program() -> bass.Bass:
    nc = bass.Bass()
    xt = nc.declare_dram_parameter("xt", [8, 128, BL], BF16, isOutput=False)
    scl = nc.declare_dram_parameter("scl", [128, KC], F32, isOutput=False)
    bia = nc.declare_dram_parameter("bia", [128, KC], F32, isOutput=False)
    wt = nc.declare_dram_parameter(
        "wt", [NG, KC2, 128, 2, 8 * 128], FP8, isOutput=False
    )
    bbp = nc.declare_dram_parameter("bb", [128, HT], F32, isOutput=False)
    wh = nc.declare_dram_parameter("wh", [HT, 128, CLASSES], BF16, isOutput=False)
    bh = nc.declare_dram_parameter("bh", [1, CLASSES], BF16, isOutput=False)
    out = nc.declare_dram_parameter("out", [BL, CLASSES], F32, isOutput=True)

    with tile.TileContext(nc) as tc:
        with (
            tc.tile_pool(name="consts", bufs=1) as consts,
            # deep wt ring so the DMA queues can run well ahead of the PE
            tc.tile_pool(name="wpool", bufs=22) as wpool,
            tc.tile_pool(name="whpool", bufs=8) as whpool,
            tc.tile_pool(name="apool", bufs=3) as apool,
            tc.tile_pool(name="opool", bufs=4) as opool,
            tc.tile_pool(name="psp", bufs=1, space="PSUM") as psp,
        ):
            # Warm the ACT table: sigmoid_and_others contains both tanh and
            # sigmoid, so the one ~2.7us table load happens under the
            # startup DMAs and never again.
            warm = consts.tile([1, 8], F32)
            nc.vector.memset(warm, 0.0)
            warm2 = consts.tile([1, 8], F32)
            nc.scalar.activation(warm2, warm, AF.Sigmoid)

            # startup-critical DMAs, spread so no queue serializes: scalar
            # gets scl/bia (the tanh-arg constants), sync and gpsimd each
            # get two of the first four xt blocks ahead of their first wt
            # chunks; the remaining xt blocks interleave into gpsimd's
            # stream inside the group-0 loop.
            scl_sb = consts.tile([128, KC], F32)
            nc.scalar.dma_start(out=scl_sb, in_=scl[:, :])
            bia_sb = consts.tile([128, KC], F32)
            nc.scalar.dma_start(out=bia_sb, in_=bia[:, :])
            xt_sb = consts.tile([128, 8, BL], BF16)
            nc.sync.dma_start(out=xt_sb[:, 0, :], in_=xt[0])
            nc.sync.dma_start(out=xt_sb[:, 1, :], in_=xt[1])
            nc.gpsimd.dma_start(out=xt_sb[:, 2, :], in_=xt[2])
            nc.gpsimd.dma_start(out=xt_sb[:, 3, :], in_=xt[3])

            # tanh basis values, computed once in group 0 and reused by
            # groups 1-3 (fp8, 64KB/partition)
            feats = consts.tile([128, KC, BL], FP8)
            phi = consts.tile([128, HT, BL], BF16)
            bb_sb = consts.tile([128, HT], F32)
            bh_sb = consts.tile([1, CLASSES], BF16)
            ones_sb = consts.tile([1, 128], BF16)
            nc.vector.memset(ones_sb, 1.0)

            wh_sb = {}

            def load_wh(i, eng):
                t = whpool.tile([128, CLASSES], BF16, name=f"wh_{i}", tag="wh")
                eng.dma_start(out=t, in_=wh[i])
                wh_sb[i] = t

            # ---- phase 1: h^T = Wb'.T-chunks @ feats (fp8 DoubleRow),
            #      phi = sigmoid(h/ALPHA + bb)
            for g in range(NG):
                psums = [
                    psp.tile([128, BL], F32, tag=f"ps{h}", name=f"ps_{g}_{h}")
                    for h in range(8)
                ]
                if g == 0:
                    # sequential order; tanh computed just-in-time.
                    # DVE precomputes the per-chunk args s*x+b so ScalarE
                    # needs only ONE big-N ACT per pair-chunk: two N=512
                    # ACTs (2*1732ns) would run slightly slower than the
                    # 8 matmuls (1728ns) and pace the PE; one N=1024 ACT
                    # (~1200ns) leaves 30% ScalarE slack.
                    for kc in range(KC2):
                        if kc in (2, 3):
                            # xt blocks 4-7, slotted after gpsimd's wt01
                            nc.gpsimd.dma_start(
                                out=xt_sb[:, 2 * kc, :], in_=xt[2 * kc]
                            )
                            nc.gpsimd.dma_start(
                                out=xt_sb[:, 2 * kc + 1, :], in_=xt[2 * kc + 1]
                            )
                        if kc == 30:
                            nc.sync.dma_start(out=bb_sb, in_=bbp[:, :])
                        if kc == 32:
                            nc.sync.dma_start(out=bh_sb, in_=bh[:, :])
                        wtile = wpool.tile([128, 2, 8 * 128], FP8, tag="wt")
                        eng = (nc.sync, nc.gpsimd, nc.scalar)[kc % 3]
                        eng.dma_start(out=wtile, in_=wt[g, kc])
                        arg = apool.tile(
                            [128, 2, BL], BF16, name=f"arg_{kc}", tag="arg"
                        )
                        for j in (0, 1):
                            m = 2 * kc + j
                            nc.vector.tensor_scalar(
                                arg[:, j, :],
                                xt_sb[:, m % 8, :],
                                scl_sb[:, m : m + 1],
                                bia_sb[:, m : m + 1],
                                mybir.AluOpType.mult,
                                mybir.AluOpType.add,
                            )
                        nc.scalar.activation(
                            feats[:, 2 * kc : 2 * kc + 2, :], arg, AF.Tanh
                        )
                        for h in range(8):
                            nc.tensor.matmul(
                                psums[h],
                                lhsT=wtile[:, :, h * 128 : (h + 1) * 128],
                                rhs=feats[:, 2 * kc : 2 * kc + 2, :],
                                start=(kc == 0),
                                stop=(kc == KC2 - 1),
                                perf_mode=DR,
                            )
                else:
                    # wavefront: bank h runs chunk s-h at slot s, so bank h's
                    # first matmul comes ~h*2us after the previous group's
                    # last — covering the previous group's sigmoid tail.
                    wt_sb = {}
                    for s in range(KC2 + 7):
                        if s < KC2:
                            wtile = wpool.tile(
                                [128, 2, 8 * 128], FP8, name=f"wt_{g}_{s}",
                                tag="wt",
                            )
                            # 3-queue rotation (ScalarE is idle in groups
                            # 1-3 apart from the 8 sigmoids, which are
                            # emitted before the first scalar-issued chunk
                            # is needed): 2 queues alone run ~4% below the
                            # PE's consumption rate and stall it on a beat.
                            if s < 8:
                                eng = nc.sync if s % 2 == 0 else nc.gpsimd
                            else:
                                eng = (nc.sync, nc.gpsimd, nc.scalar)[s % 3]
                            eng.dma_start(out=wtile, in_=wt[g, s])
                            wt_sb[s] = wtile
                        elif g == 3:
                            # prefetch the first 8 Wh tiles in the wavefront
                            # tail slots (s=64..71): the wt stream is done,
                            # so these 2MB don't compete with it. The
                            # tile_wait_until anchor stops the scheduler
                            # from hoisting these no-dep DMAs to startup.
                            with tc.tile_wait_until(0.21):
                                load_wh(
                                    s - KC2,
                                    nc.gpsimd if s % 2 == 0 else nc.sync,
                                )
                        for h in range(8):
                            kc = s - h
                            if 0 <= kc < KC2:
                                nc.tensor.matmul(
                                    psums[h],
                                    lhsT=wt_sb[kc][:, :, h * 128 : (h + 1) * 128],
                                    rhs=feats[:, 2 * kc : 2 * kc + 2, :],
                                    start=(kc == 0),
                                    stop=(kc == KC2 - 1),
                                    perf_mode=DR,
                                )
                for h in range(8):
                    i = g * 8 + h
                    nc.scalar.activation(
                        phi[:, i, :],
                        psums[h],
                        AF.Sigmoid,
                        bias=bb_sb[:, i : i + 1],
                        scale=1.0 / ALPHA,
                    )

            # ---- phase 2: logits = phi.T @ Wh + bh  (bf16, single pass,
            #      all 8 psum banks, Wh streamed through an 8-deep ring)
            ps2 = {}
            for b4 in range(4):
                for c2 in range(2):
                    idx = b4 * 2 + c2
                    ps = psp.tile(
                        [128, CH], F32, tag=f"ps{idx}", name=f"q_{b4}_{c2}"
                    )
                    ps2[(b4, c2)] = ps
                    # bias init: psum = ones^T @ bh_slice (K=1 matmul)
                    nc.tensor.matmul(
                        ps,
                        lhsT=ones_sb,
                        rhs=bh_sb[:, c2 * CH : (c2 + 1) * CH],
                        start=True,
                        stop=False,
                    )
            for i in range(HT):
                if i + 7 < HT:
                    load_wh(i + 7, nc.sync if i % 2 == 0 else nc.gpsimd)
                for b4 in range(4):
                    for c2 in range(2):
                        nc.tensor.matmul(
                            ps2[(b4, c2)],
                            lhsT=phi[:, i, b4 * 128 : (b4 + 1) * 128],
                            rhs=wh_sb[i][:, c2 * CH : (c2 + 1) * CH],
                            start=False,
                            stop=(i == HT - 1),
                        )
            # eviction tail: copy each psum half out on alternating engines
            # (DVE + ScalarE) and DMA each half separately over 3 queues so
            # the tail after the last matmul is a few us, not 15.
            for b4 in range(4):
                out_sb = opool.tile(
                    [128, CLASSES], F32, name=f"out_sb_{b4}", tag="out"
                )
                for c2 in range(2):
                    p = b4 * 2 + c2
                    dst = out_sb[:, c2 * CH : (c2 + 1) * CH]
                    if p % 2 == 0:
                        nc.vector.tensor_copy(dst, ps2[(b4, c2)])
                    else:
                        nc.scalar.activation(dst, ps2[(b4, c2)], AF.Copy)
                    (nc.sync, nc.gpsimd, nc.scalar)[p % 3].dma_start(
                        out=out[
                            b4 * 128 : (b4 + 1) * 128,
                            c2 * CH : (c2 + 1) * CH,
                        ],
                        in_=dst,
                    )
    return nc


_CACHE: dict = {}


def _prep_inputs(x, centers, scales, Wb, bb, Wh, bh):
    bf16 = ml_dtypes.bfloat16
    fp8 = ml_dtypes.float8_e4m3
    # K reorder: k' = j*IN_DIM + d  (so a 128-chunk shares one (d-block, j))
    scale_vec = np.ascontiguousarray(scales.T).reshape(K)
    bias_vec = np.ascontiguousarray(-(scales * centers).T).reshape(K)
    scl = np.ascontiguousarray(scale_vec.reshape(KC, 128).T).astype(np.float32)
    bia = np.ascontiguousarray(bias_vec.reshape(KC, 128).T).astype(np.float32)
    # Wb rows permuted to k' order, scaled by ALPHA, tiled
    # [g, kc, ki, pair, h*128+c] for DoubleRow (pair = second 128 of each
    # 256-deep contraction chunk)
    Wbp = (
        Wb.reshape(IN_DIM, NBASIS, HIDDEN)
        .transpose(1, 0, 2)
        .reshape(K, HIDDEN)
    )
    wt = np.ascontiguousarray(
        (Wbp * ALPHA)
        .reshape(KC2, 2, 128, NG, 8, 128)
        .transpose(3, 0, 2, 1, 4, 5)
        .reshape(NG, KC2, 128, 2, 8 * 128)
    ).astype(fp8)
    bbp = np.ascontiguousarray(bb.reshape(HT, 128).T).astype(np.float32)
    whp = np.ascontiguousarray(Wh.reshape(HT, 128, CLASSES)).astype(bf16)
    bhp = np.ascontiguousarray(bh.reshape(1, CLASSES)).astype(bf16)
    xT = np.ascontiguousarray(x.T)  # [IN_DIM, B]
    in_maps = []
    for c in range(NCORES):
        xt_c = (
            np.ascontiguousarray(xT[:, c * BL : (c + 1) * BL])
            .reshape(8, 128, BL)
            .astype(bf16)
        )
        in_maps.append(
            {
                "xt": xt_c,
                "scl": scl,
                "bia": bia,
                "wt": wt,
                "bb": bbp,
                "wh": whp,
                "bh": bhp,
            }
        )
    return in_maps


def kernel(x, centers, scales, Wb, bb, Wh, bh):
    x = np.asarray(x, dtype=np.float32)
    centers = np.asarray(centers, dtype=np.float32)
    scales = np.asarray(scales, dtype=np.float32)
    Wb = np.asarray(Wb, dtype=np.float32)
    bb = np.asarray(bb, dtype=np.float32)
    Wh = np.asarray(Wh, dtype=np.float32)
    bh = np.asarray(bh, dtype=np.float32)

    if "nc" not in _CACHE:
        _CACHE["nc"] = build_program()
    nc = _CACHE["nc"]
    in_maps = _prep_inputs(x, centers, scales, Wb, bb, Wh, bh)
    res = run_bass_kernel_spmd(nc, in_maps, list(range(NCORES)))
    return np.concatenate(
        [res.results[c]["out"] for c in range(NCORES)], axis=0
    )


# revision 25
# speedup vs baseline: 1.0056x; 1.0021x over previous
"""Trainium2 Bass kernel for the KAN classifier (tanh-basis MLP).

logits = sigmoid(tanh((x[:,:,None]-centers)*scales).reshape(B,-1) @ Wb + bb) @ Wh + bh

Sharding: data-parallel over batch across 8 NeuronCores (512 rows each).
Per core: basis expansion on ScalarE (tanh LUT with per-partition scale/bias,
fp8 output, computed once in group 0 and cached in SBUF), the big matmul on
TensorE in fp8 DoubleRow mode (256-deep contraction per matmul, ~2x PE rate),
the head matmul in bf16 with fp32 PSUM accumulation.

Groups 1-3 issue their matmuls in a skewed "wavefront" order so that PSUM
bank h's first matmul trails bank h-1 by one chunk: the 8 serial sigmoid
evictions of the previous group then stay off the PE critical path.
"""

import sys

sys.path.insert(0, "/opt/trn_rl_repo")

import ml_dtypes
import numpy as np

import concourse.bass as bass
import concourse.mybir as mybir
import concourse.tile as tile
from concourse.bass_utils import run_bass_kernel_spmd
from concourse.vector_clock import ScopedClock

IN_DIM, HIDDEN, CLASSES, NBASIS, B = 1024, 4096, 1000, 16, 4096
NCORES = 8
BL = B // NCORES          # 512 batch rows per core
K = IN_DIM * NBASIS       # 16384 contraction dim (reordered j*IN_DIM + d)
KC = K // 128             # 128 K-chunks (tanh/scale/bias granularity)
KC2 = KC // 2             # 64 K-pair-chunks (one DoubleRow matmul each)
NG = 4                    # hidden groups (8 psum banks each)
HT = HIDDEN // 128        # 32 hidden tiles
CH = CLASSES // 2         # 500 logits per psum half
ALPHA = 2048.0            # fp8 weight scale (undone in the sigmoid)

F32 = mybir.dt.float32
BF16 = mybir.dt.bfloat16
FP8 = mybir.dt.float8e4
AF = mybir.ActivationFunctionType
DR = mybir.MatmulPerfMode.DoubleRow


def _patched_drain_and_barrier(self, tick_clock, wait_clock):
    # The walrus build in this image caps sync-waits per CTRL instruction;
    # stock Tile piles one wait per live semaphore onto the single tail
    # Drain. Re-emit them as standalone single-wait instructions.
    nc = self.nc
    drain_inst = nc.sync.drain()
    wait_clock.add_sem_waits(
        drain_inst.ins, ScopedClock({None: tick_clock.global_clock})
    )
    si = drain_inst.ins.sync_info
    waits = list(si.on_wait)
    if len(waits) > 2:
        si.on_wait = []
        handles = {h.num: h for h in self.sems.allocated().values()}
        for w in waits:
            nc.sync.wait_ge(handles[w.id], w.wait_value)
    nc.all_engine_barrier()
    popped = nc._tile_sem_poison_stack.pop()
    assert popped is self._sem_poison
    nc.clear_and_free_semaphores(list(self.sems.allocated().values()))
    nc.all_engine_barrier()


tile.TileContext._drain_and_barrier = _patched_drain_and_barrier

# Walrus also rejects >2 sync-waits on ANY instruction. Post-process the
# serialized BIR: hoist excess waits onto EventSemaphore instructions emitted
# immediately before, on the same engine (engine streams are in-order, so a
# prior standalone wait is equivalent).
_MAXW = 1


def _split_excess_waits(raw: bytes) -> bytes:
    import orjson

    m = orjson.loads(raw)
    n_new = 0
    for fn in m.get("functions", []):
        for bb in fn.get("blocks", []):
            insts = bb.get("instructions", [])
            if not any(
                len((i.get("sync_info") or {}).get("on_wait") or []) > _MAXW
                for i in insts
            ):
                continue
            out = []
            for ins in insts:
                si = ins.get("sync_info")
                ow = (si or {}).get("on_wait") or []
                if len(ow) > _MAXW:
                    imm = [w for w in ow if not w.get("wait_reg")]
                    reg = [w for w in ow if w.get("wait_reg")]
                    assert len(reg) <= _MAXW, "too many register waits"
                    n_hoist = len(ow) - _MAXW
                    hoisted, kept = imm[:n_hoist], imm[n_hoist:] + reg
                    for w in hoisted:
                        n_new += 1
                        out.append(
                            {
                                "debug": ins.get("debug"),
                                "engine": ins["engine"],
                                "ins": [],
                                "name": f"WSPLIT-{n_new}",
                                "opcode": "EventSemaphore",
                                "outs": [],
                                "sync_info": {"on_update": [], "on_wait": [w]},
                            }
                        )
                    si["on_wait"] = kept
                out.append(ins)
            bb["instructions"] = out
    return orjson.dumps(m)


_orig_to_json_bytes = bass.Bass.to_json_bytes


def _to_json_bytes_split(self, *a, **kw):
    return _split_excess_waits(_orig_to_json_bytes(self, *a, **kw))


bass.Bass.to_json_bytes = _to_json_bytes_split


def build_program() -> bass.Bass:
    nc = bass.Bass()
    xt = nc.declare_dram_parameter("xt", [8, 128, BL], BF16, isOutput=False)
    scl = nc.declare_dram_parameter("scl", [128, KC], F32, isOutput=False)
    bia = nc.declare_dram_parameter("bia", [128, KC], F32, isOutput=False)
    wt = nc.declare_dram_parameter(
        "wt", [NG, KC2, 128, 2, 8 * 128], FP8, isOutput=False
    )
    bbp = nc.declare_dram_parameter("bb", [128, HT], F32, isOutput=False)
    wh = nc.declare_dram_parameter("wh", [HT, 128, CLASSES], BF16, isOutput=False)
    bh = nc.declare_dram_parameter("bh", [1, CLASSES], BF16, isOutput=False)
    out = nc.declare_dram_parameter("out", [BL, CLASSES], F32, isOutput=True)

    with tile.TileContext(nc) as tc:
        with (
            tc.tile_pool(name="consts", bufs=1) as consts,
            # deep wt ring so the DMA queues can run well ahead of the PE
            tc.tile_pool(name="wpool", bufs=22) as wpool,
            tc.tile_pool(name="whpool", bufs=8) as whpool,
            tc.tile_pool(name="apool", bufs=3) as apool,
            tc.tile_pool(name="opool", bufs=4) as opool,
            tc.tile_pool(name="psp", bufs=1, space="PSUM") as psp,
        ):
            # Warm the ACT table: sigmoid_and_others contains both tanh and
            # sigmoid, so the one ~2.7us table load happens under the
            # startup DMAs and never again.
            warm = consts.tile([1, 8], F32)
            nc.vector.memset(warm, 0.0)
            warm2 = consts.tile([1, 8], F32)
            nc.scalar.activation(warm2, warm, AF.Sigmoid)

            # startup-critical DMAs, spread so no queue serializes: scalar
            # gets scl/bia (the tanh-arg constants), sync and gpsimd each
            # get two of the first four xt blocks ahead of their first wt
            # chunks; the remaining xt blocks interleave into gpsimd's
            # stream inside the group-0 loop.
            # scl/bia split into a tiny head + rest: the head completes
            # instantly even when the big wt transfers hog the DMA engines,
            # so the first tanh-arg ops aren't blocked on a full-tile DMA
            # whose tail packets finish ~4us late.
            scl_sb = consts.tile([128, KC], F32)
            nc.scalar.dma_start(out=scl_sb[:, 0:8], in_=scl[:, 0:8])
            bia_sb = consts.tile([128, KC], F32)
            nc.scalar.dma_start(out=bia_sb[:, 0:8], in_=bia[:, 0:8])
            nc.scalar.dma_start(out=scl_sb[:, 8:KC], in_=scl[:, 8:KC])
            nc.scalar.dma_start(out=bia_sb[:, 8:KC], in_=bia[:, 8:KC])
            xt_sb = consts.tile([128, 8, BL], BF16)
            nc.sync.dma_start(out=xt_sb[:, 0, :], in_=xt[0])
            nc.sync.dma_start(out=xt_sb[:, 1, :], in_=xt[1])
            nc.gpsimd.dma_start(out=xt_sb[:, 2, :], in_=xt[2])
            nc.gpsimd.dma_start(out=xt_sb[:, 3, :], in_=xt[3])

            # tanh basis values, computed once in group 0 and reused by
            # groups 1-3 (fp8, 64KB/partition)
            feats = consts.tile([128, KC, BL], FP8)
            phi = consts.tile([128, HT, BL], BF16)
            bb_sb = consts.tile([128, HT], F32)
            bh_sb = consts.tile([1, CLASSES], BF16)
            ones_sb = consts.tile([1, 128], BF16)
            nc.vector.memset(ones_sb, 1.0)

            wh_sb = {}

            def load_wh(i, eng):
                t = whpool.tile([128, CLASSES], BF16, name=f"wh_{i}", tag="wh")
                eng.dma_start(out=t, in_=wh[i])
                wh_sb[i] = t

            # ---- phase 1: h^T = Wb'.T-chunks @ feats (fp8 DoubleRow),
            #      phi = sigmoid(h/ALPHA + bb)
            for g in range(NG):
                psums = [
                    psp.tile([128, BL], F32, tag=f"ps{h}", name=f"ps_{g}_{h}")
                    for h in range(8)
                ]
                if g == 0:
                    # sequential order; tanh computed just-in-time.
                    # DVE precomputes the per-chunk args s*x+b so ScalarE
                    # needs only ONE big-N ACT per pair-chunk: two N=512
                    # ACTs (2*1732ns) would run slightly slower than the
                    # 8 matmuls (1728ns) and pace the PE; one N=1024 ACT
                    # (~1200ns) leaves 30% ScalarE slack.
                    for kc in range(KC2):
                        if kc in (2, 3):
                            # xt blocks 4-7, slotted after gpsimd's wt01
                            nc.gpsimd.dma_start(
                                out=xt_sb[:, 2 * kc, :], in_=xt[2 * kc]
                            )
                            nc.gpsimd.dma_start(
                                out=xt_sb[:, 2 * kc + 1, :], in_=xt[2 * kc + 1]
                            )
                        if kc == 30:
                            nc.sync.dma_start(out=bb_sb, in_=bbp[:, :])
                        if kc == 32:
                            nc.sync.dma_start(out=bh_sb, in_=bh[:, :])
                        wtile = wpool.tile([128, 2, 8 * 128], FP8, tag="wt")
                        eng = (nc.sync, nc.gpsimd, nc.scalar)[kc % 3]
                        eng.dma_start(out=wtile, in_=wt[g, kc])
                        arg = apool.tile(
                            [128, 2, BL], BF16, name=f"arg_{kc}", tag="arg"
                        )
                        for j in (0, 1):
                            m = 2 * kc + j
                            nc.vector.tensor_scalar(
                                arg[:, j, :],
                                xt_sb[:, m % 8, :],
                                scl_sb[:, m : m + 1],
                                bia_sb[:, m : m + 1],
                                mybir.AluOpType.mult,
                                mybir.AluOpType.add,
                            )
                        nc.scalar.activation(
                            feats[:, 2 * kc : 2 * kc + 2, :], arg, AF.Tanh
                        )
                        for h in range(8):
                            nc.tensor.matmul(
                                psums[h],
                                lhsT=wtile[:, :, h * 128 : (h + 1) * 128],
                                rhs=feats[:, 2 * kc : 2 * kc + 2, :],
                                start=(kc == 0),
                                stop=(kc == KC2 - 1),
                                perf_mode=DR,
                            )
                else:
                    # wavefront: bank h runs chunk s-h at slot s, so bank h's
                    # first matmul comes ~h*2us after the previous group's
                    # last — covering the previous group's sigmoid tail.
                    wt_sb = {}
                    for s in range(KC2 + 7):
                        if s < KC2:
                            wtile = wpool.tile(
                                [128, 2, 8 * 128], FP8, name=f"wt_{g}_{s}",
                                tag="wt",
                            )
                            # 3-queue rotation (ScalarE is idle in groups
                            # 1-3 apart from the 8 sigmoids, which are
                            # emitted before the first scalar-issued chunk
                            # is needed): 2 queues alone run ~4% below the
                            # PE's consumption rate and stall it on a beat.
                            if s < 8:
                                eng = nc.sync if s % 2 == 0 else nc.gpsimd
                            else:
                                eng = (nc.sync, nc.gpsimd, nc.scalar)[s % 3]
                            eng.dma_start(out=wtile, in_=wt[g, s])
                            wt_sb[s] = wtile
                        elif g == 3:
                            # prefetch the first 8 Wh tiles in the wavefront
                            # tail slots (s=64..71): the wt stream is done,
                            # so these 2MB don't compete with it. The
                            # tile_wait_until anchor stops the scheduler
                            # from hoisting these no-dep DMAs to startup.
                            with tc.tile_wait_until(0.21):
                                load_wh(
                                    s - KC2,
                                    nc.gpsimd if s % 2 == 0 else nc.sync,
                                )
                        for h in range(8):
                            kc = s - h
                            if 0 <= kc < KC2:
                                nc.tensor.matmul(
                                    psums[h],
                                    lhsT=wt_sb[kc][:, :, h * 128 : (h + 1) * 128],
                                    rhs=feats[:, 2 * kc : 2 * kc + 2, :],
                                    start=(kc == 0),
                                    stop=(kc == KC2 - 1),
                                    perf_mode=DR,
                                )
                for h in range(8):
                    i = g * 8 + h
                    nc.scalar.activation(
                        phi[:, i, :],
                        psums[h],
                        AF.Sigmoid,
                        bias=bb_sb[:, i : i + 1],
                        scale=1.0 / ALPHA,
                    )

            # ---- phase 2: logits = phi.T @ Wh + bh  (bf16, single pass,
            #      all 8 psum banks, Wh streamed through an 8-deep ring)
            ps2 = {}
            for b4 in range(4):
                for c2 in range(2):
                    idx = b4 * 2 + c2
                    ps = psp.tile(
                        [128, CH], F32, tag=f"ps{idx}", name=f"q_{b4}_{c2}"
                    )
                    ps2[(b4, c2)] = ps
                    # bias init: psum = ones^T @ bh_slice (K=1 matmul)
                    nc.tensor.matmul(
                        ps,
                        lhsT=ones_sb,
                        rhs=bh_sb[:, c2 * CH : (c2 + 1) * CH],
                        start=True,
                        stop=False,
                    )
            for i in range(HT):
                if i + 7 < HT:
                    load_wh(i + 7, nc.sync if i % 2 == 0 else nc.gpsimd)
                for b4 in range(4):
                    for c2 in range(2):
                        nc.tensor.matmul(
                            ps2[(b4, c2)],
                            lhsT=phi[:, i, b4 * 128 : (b4 + 1) * 128],
                            rhs=wh_sb[i][:, c2 * CH : (c2 + 1) * CH],
                            start=False,
                            stop=(i == HT - 1),
                        )
            # eviction tail: copy each psum half out on alternating engines
            # (DVE + ScalarE) and DMA each half separately over 3 queues so
            # the tail after the last matmul is a few us, not 15.
            for b4 in range(4):
                out_sb = opool.tile(
                    [128, CLASSES], F32, name=f"out_sb_{b4}", tag="out"
                )
                for c2 in range(2):
                    p = b4 * 2 + c2
                    dst = out_sb[:, c2 * CH : (c2 + 1) * CH]
                    if p % 2 == 0:
                        nc.vector.tensor_copy(dst, ps2[(b4, c2)])
                    else:
                        nc.scalar.activation(dst, ps2[(b4, c2)], AF.Copy)
                    (nc.sync, nc.gpsimd, nc.scalar)[p % 3].dma_start(
                        out=out[
                            b4 * 128 : (b4 + 1) * 128,
                            c2 * CH : (c2 + 1) * CH,
                        ],
                        in_=dst,
                    )
    return nc


_CACHE: dict = {}


def _prep_inputs(x, centers, scales, Wb, bb, Wh, bh):
    bf16 = ml_dtypes.bfloat16
    fp8 = ml_dtypes.float8_e4m3
    # K reorder: k' = j*IN_DIM + d  (so a 128-chunk shares one (d-block, j))
    scale_vec = np.ascontiguousarray(scales.T).reshape(K)
    bias_vec = np.ascontiguousarray(-(scales * centers).T).reshape(K)
    scl = np.ascontiguousarray(scale_vec.reshape(KC, 128).T).astype(np.float32)
    bia = np.ascontiguousarray(bias_vec.reshape(KC, 128).T).astype(np.float32)
    # Wb rows permuted to k' order, scaled by ALPHA, tiled
    # [g, kc, ki, pair, h*128+c] for DoubleRow (pair = second 128 of each
    # 256-deep contraction chunk)
    Wbp = (
        Wb.reshape(IN_DIM, NBASIS, HIDDEN)
        .transpose(1, 0, 2)
        .reshape(K, HIDDEN)
    )
    wt = np.ascontiguousarray(
        (Wbp * ALPHA)
        .reshape(KC2, 2, 128, NG, 8, 128)
        .transpose(3, 0, 2, 1, 4, 5)
        .reshape(NG, KC2, 128, 2, 8 * 128)
    ).astype(fp8)
    bbp = np.ascontiguousarray(bb.reshape(HT, 128).T).astype(np.float32)
    whp = np.ascontiguousarray(Wh.reshape(HT, 128, CLASSES)).astype(bf16)
    bhp = np.ascontiguousarray(bh.reshape(1, CLASSES)).astype(bf16)
    xT = np.ascontiguousarray(x.T)  # [IN_DIM, B]
    in_maps = []
    for c in range(NCORES):
        xt_c = (
            np.ascontiguousarray(xT[:, c * BL : (c + 1) * BL])
            .reshape(8, 128, BL)
            .astype(bf16)
        )
        in_maps.append(
            {
                "xt": xt_c,
                "scl": scl,
                "bia": bia,
                "wt": wt,
                "bb": bbp,
                "wh": whp,
                "bh": bhp,
            }
        )
    return in_maps


def kernel(x, centers, scales, Wb, bb, Wh, bh):
    x = np.asarray(x, dtype=np.float32)
    centers = np.asarray(centers, dtype=np.float32)
    scales = np.asarray(scales, dtype=np.float32)
    Wb = np.asarray(Wb, dtype=np.float32)
    bb = np.asarray(bb, dtype=np.float32)
    Wh = np.asarray(Wh, dtype=np.float32)
    bh = np.asarray(bh, dtype=np.float32)

    if "nc" not in _CACHE:
        _CACHE["nc"] = build_program()
    nc = _CACHE["nc"]
    in_maps = _prep_inputs(x, centers, scales, Wb, bb, Wh, bh)
    res = run_bass_kernel_spmd(nc, in_maps, list(range(NCORES)))
    return np.concatenate(
        [res.results[c]["out"] for c in range(NCORES)], axis=0
    )
